# revision 37
# baseline (speedup 1.0000x reference)
"""BiMamba layer Trainium2 kernel (8 NeuronCores, SPMD).

Sharding: 4 batch-groups x 2 d_inner-halves. Core (g, h) handles the 3
(b*f) scan units of batch g for d_inner channels [96h, 96h+96), both scan
directions. Each core emits a partial out-projection; the host sums the
two halves per batch and adds out_proj_b.

Channel permutation: conv/in_proj output channels are permuted host-side
so the core's own 96 channels occupy xc0 rows 0:96 — u is a slice of the
conv output, no duplicate conv group.

All 36 (d,n)-row-tiles stream through one software-pipelined loop with a
2-deep skew so no engine queue head-blocks on another engine:
  A(t): delta/du bcast DMA (HWDGE) -> daf=Exp (ACT) -> dbu (Pool TT)
  B(t-1): fwd/bwd scans + hs=hf+rev(hb) (DVE)
  C(t-2): hc=hs*C (DVE even / Pool odd) -> n-reduction matmuls (PE)
Unit preambles (conv+in_proj+silu, x_proj, dt_proj+softplus, du) and the
previous unit's out-projection are emitted at fixed hooks inside the
stream; activations are batched by act-table set to avoid reloads.
"""
from contextlib import ExitStack

import numpy as np

import concourse.bass as bass
import concourse.tile as tile
from concourse import bacc, mybir
from concourse.bass_utils import run_bass_kernel_spmd

F32 = mybir.dt.float32
FR = mybir.dt.float32r
BF = mybir.dt.bfloat16
AF = mybir.ActivationFunctionType
OP = mybir.AluOpType

B, SEQ, DIM = 4, 6144, 384
L = 2048                  # per-unit sequence length
NU = 3                    # units per core
DIN, DH, NST, DTR = 192, 96, 16, 24
NK = 12                   # (DH*NST)//128 row-tiles
LC = 512                  # psum column chunk
NLC = L // LC
NCORES = 8
NT = NU * NK              # global tile count
SKEW_B, SKEW_C = 1, 3

_NC_CACHE = {}


def _patch_act_tables():
    """Confine Exp/Ln to the combined natural_log_exp table so the act-table
    chooser never alternates between the separate exp / ln tables."""
    import concourse.hw_specs as hw_specs
    if getattr(hw_specs.get_activation_tables, "_bimamba_patched", False):
        return
    orig = hw_specs.get_activation_tables

    def patched(module_arch):
        tables = dict(orig(module_arch))
        import concourse.mybir as mb
        aft = mb.ActivationFunctionType
        for name, funcs in tables.items():
            if name != "natural_log_exp_and_others":
                tables[name] = funcs - {aft.Exp, aft.Ln}
        return tables

    patched._bimamba_patched = True
    hw_specs.get_activation_tables = patched
    import concourse.bacc as bacc_mod
    bacc_mod.get_activation_tables = patched


def _build(ab_same: bool):
    _patch_act_tables()
    nc = bacc.Bacc("TRN2", target_bir_lowering=False, debug=False)

    def din(name, shape, dt=F32):
        return nc.dram_tensor(name, list(shape), dt, kind="ExternalInput").ap()

    xtp_d = din("xtp", (NU, 3, 128, L + 2), BF)
    wm_d = din("wm", (9, 128, DIN), BF)
    efix_d = din("efix", (1, 2, DIN))
    one_d = din("one", (1, 1))
    bsil_d = din("bsil", (128, 2))
    wdd_d = din("wdd", (128, DH))
    wdd2b_d = din("wdd2b", (64, DH), BF)
    wxpbc_d = din("wxpbc", (DIN, 256))
    wxpbc2b_d = din("wxpbc2b", (64, 256), BF)
    bsp_d = din("bsp", (DH, 1))
    acol_d = din("acol", (NK * 128,))
    abcol_d = din("abcol", (NK * 128,))
    seln_d = din("seln", (4, 128, 32), BF)
    ddiag_d = din("ddiag", (DH, DH))
    wouty_d = din("wouty", (DH, DIM))
    woutz_d = din("woutz", (DH, DIM))
    out_d = nc.dram_tensor("out", [NU, L, DIM], F32, kind="ExternalOutput").ap()
    outt_d = nc.dram_tensor("out_tail", [L, DIM], F32,
                            kind="ExternalOutput").ap()

    abufs = 4 if ab_same else 2

    with tile.TileContext(nc) as tc, ExitStack() as ctx:
        cp = ctx.enter_context(tc.tile_pool(name="consts", bufs=1))
        px = ctx.enter_context(tc.tile_pool(name="px", bufs=1))
        pxc = ctx.enter_context(tc.tile_pool(name="pxc", bufs=2))
        psm = ctx.enter_context(tc.tile_pool(name="psm", bufs=2))
        psm1 = ctx.enter_context(tc.tile_pool(name="psm1", bufs=1))
        pbig = ctx.enter_context(tc.tile_pool(name="pbig", bufs=2))
        pout = ctx.enter_context(tc.tile_pool(name="pout", bufs=4))
        ppa = ctx.enter_context(tc.tile_pool(name="ppa", bufs=2, space="PSUM"))
        ppy = ctx.enter_context(tc.tile_pool(name="ppy", bufs=4, space="PSUM"))

        state = {}

        def emit_xt_chunk(u, c):
            if c == 0:
                state[u] = {"xt": px.tile([128, 3, L + 2], BF, name="xt")}
            xt = state[u]["xt"]
            c0 = c * LC
            c1 = min(L + 2, c0 + LC + 2)
            nc.sync.dma_start(
                xt[:, :, c0:c1],
                xtp_d[u][:, :, c0:c1].transpose([1, 0, 2]))

        # ---- xt(0) first, then constants ordered by first use ----
        warm = cp.tile([1, 2], F32)
        nc.gpsimd.memset(warm[:], 0.0)
        nc.gpsimd.tensor_tensor(warm[:, 0:1], warm[:, 0:1], warm[:, 1:2],
                                OP.mult)
        wps = ppa.tile([1, 64], F32, tag="ppa", name="wps")
        for i in range(20):
            nc.tensor.matmul(wps[:], warm[0:1, 0:1], warm[0:1, 0:1]
                             .broadcast_to([1, 64]), start=(i == 0),
                             stop=(i == 19))
        for c in range(NLC):
            emit_xt_chunk(0, c)
        wm_sb = cp.tile([128, 9, DIN], BF)
        nc.sync.dma_start(wm_sb[:], wm_d.transpose([1, 0, 2]))
        efix_sb = cp.tile([1, 2, DIN], F32)
        nc.sync.dma_start(efix_sb[:], efix_d)
        one_sb = cp.tile([1, 1], F32)
        nc.sync.dma_start(one_sb[:], one_d)
        bsil_sb = cp.tile([128, 2], F32)
        nc.sync.dma_start(bsil_sb[:], bsil_d)
        wdd_sb = cp.tile([128, DH], FR)
        nc.sync.dma_start(wdd_sb[:], wdd_d.bitcast(FR))
        wdd2_sb = cp.tile([64, DH], BF)
        nc.sync.dma_start(wdd2_sb[:], wdd2b_d)
        bsp_sb = cp.tile([DH, 1], F32)
        nc.sync.dma_start(bsp_sb[:], bsp_d)
        wxpbc_sb = cp.tile([128, 256], FR)
        nc.sync.dma_start(wxpbc_sb[:], wxpbc_d[0:128, :].bitcast(FR))
        wxpbc2_sb = cp.tile([64, 256], BF)
        nc.sync.dma_start(wxpbc2_sb[:], wxpbc2b_d)
        acol_sb = cp.tile([128, NK], F32)
        nc.sync.dma_start(acol_sb[:], acol_d.rearrange("(k p) -> p k", p=128))
        abcol_sb = cp.tile([128, NK], F32)
        nc.sync.dma_start(abcol_sb[:], abcol_d.rearrange("(k p) -> p k", p=128))
        seln_sb = cp.tile([128, 4, 32], BF)
        nc.sync.dma_start(seln_sb[:], seln_d.transpose([1, 0, 2]))
        ddiag_sb = cp.tile([DH, DH], FR)
        nc.sync.dma_start(ddiag_sb[:], ddiag_d.bitcast(FR))
        wouty_sb = cp.tile([DH, DIM], FR)
        nc.sync.dma_start(wouty_sb[:], wouty_d.bitcast(FR))
        woutz_sb = cp.tile([DH, DIM], FR)
        nc.sync.dma_start(woutz_sb[:], woutz_d.bitcast(FR))

        def alloc_unit(u):
            st = state[u]
            st["xc"] = (pxc.tile([128, L], FR, name="xc0"),
                        pxc.tile([64, L], BF, name="xc1", bufs=1))
            st["brep"] = psm.tile([128, L], BF, name="brep")
            st["crep"] = psm.tile([128, L], BF, name="crep")
            st["ddu"] = psm.tile([DH, 2, L], BF, name="ddu")
            st["ubf"] = psm.tile([DH, L], BF, name="ubf", bufs=1)
            st["y"] = psm.tile([DH, L], FR, name="y_sb")

        def emit_conv_lc(u, lc, ctag="ppa", cbufs=2):
            xc0, xc1 = state[u]["xc"]
            xt = state[u]["xt"]
            sl = slice(lc * LC, (lc + 1) * LC)
            for c0, cw, dst, bias_ap in [
                    (0, 128, xc0, bsil_sb[0:128, 0:1]),
                    (128, 64, xc1, bsil_sb[0:64, 1:2])]:
                pool = ppy if ctag == "pys" else ppa
                ps = pool.tile([128, LC], F32, tag=ctag, name="ps_conv",
                               bufs=cbufs)
                mms = []
                for s in range(3):
                    for kt in range(3):
                        mms.append((ps[0:cw, :],
                                    wm_sb[:, s * 3 + kt, c0:c0 + cw],
                                    xt[:, kt, s + lc * LC:s + lc * LC + LC]))
                if lc == 0:
                    mms.append((ps[0:cw, 0:1],
                                efix_sb[0:1, 0, c0:c0 + cw], one_sb[:]))
                if lc == NLC - 1:
                    mms.append((ps[0:cw, LC - 1:LC],
                                efix_sb[0:1, 1, c0:c0 + cw], one_sb[:]))
                for i, (o, lh, rh) in enumerate(mms):
                    nc.tensor.matmul(o, lh, rh, start=(i == 0),
                                     stop=(i == len(mms) - 1))
                nc.scalar.activation(dst[:, sl], ps[0:cw, :], AF.Silu,
                                     bias=bias_ap)

        def emit_dt_lc(u, lc):
            """dt x_proj + dt_proj + softplus + du + B for one chunk."""
            st = state[u]
            xc0, xc1 = st["xc"]
            sl = slice(lc * LC, (lc + 1) * LC)
            pdp = ppa.tile([128, LC], F32, tag="ppa", name="ps_dp")
            nc.tensor.matmul(pdp[0:DH, :], wdd_sb[:], xc0[:, sl],
                             start=True, stop=False)
            nc.tensor.matmul(pdp[0:DH, :], wdd2_sb[:], xc1[:, sl],
                             start=False, stop=True)
            nc.scalar.activation(st["ddu"][:, 1, sl], pdp[0:DH, :], AF.Exp,
                                 bias=bsp_sb[:])
            nc.scalar.activation(st["ddu"][:, 0, sl], st["ddu"][:, 1, sl],
                                 AF.Ln, bias=1.0)
            nc.scalar.activation(st["ubf"][:, sl], st["xc"][0][0:DH, sl],
                                 AF.Copy)
            nc.vector.tensor_tensor(st["ddu"][:, 1, sl], st["ddu"][:, 0, sl],
                                    st["ubf"][:, sl], OP.mult)
            pbr = ppa.tile([128, LC], F32, tag="ps_o", name="ps_br")
            nc.tensor.matmul(pbr[:], wxpbc_sb[:, 0:128], xc0[:, sl],
                             start=True, stop=False)
            nc.tensor.matmul(pbr[:], wxpbc2_sb[:, 0:128], xc1[:, sl],
                             start=False, stop=True)
            nc.scalar.activation(st["brep"][:, sl], pbr[:], AF.Copy)

        def emit_cr(u):
            st = state[u]
            xc0, xc1 = st["xc"]
            for lc in range(NLC):
                sl = slice(lc * LC, (lc + 1) * LC)
                pcr = ppa.tile([128, LC], F32, tag="ppa", name="ps_cr")
                nc.tensor.matmul(pcr[:], wxpbc_sb[:, 128:256], xc0[:, sl],
                                 start=True, stop=False)
                nc.tensor.matmul(pcr[:], wxpbc2_sb[:, 128:256], xc1[:, sl],
                                 start=False, stop=True)
                nc.scalar.activation(st["crep"][:, sl], pcr[:], AF.Copy)

        def emit_pre0():
            alloc_unit(0)
            emit_conv_lc(0, 0, ctag="pys", cbufs=4)
            emit_conv_lc(0, 1, ctag="pys", cbufs=4)
            emit_dt_lc(0, 0)
            emit_conv_lc(0, 2, ctag="pys", cbufs=4)
            emit_dt_lc(0, 1)
            emit_conv_lc(0, 3, ctag="pys", cbufs=4)
            emit_dt_lc(0, 2)
            emit_dt_lc(0, 3)

        def emit_conv(u):
            alloc_unit(u)
            for lc in range(NLC):
                emit_conv_lc(u, lc, ctag=("ppa" if lc % 2 == 0 else "ps_o"))



        def stage_a(u, k):
            st = state[u]
            rep = pbig.tile([128, 2, L], BF, bufs=3)
            nc.sync.dma_start(
                rep[:],
                st["ddu"][8 * k:8 * k + 8, :, :].unsqueeze(1)
                .broadcast_to([8, 16, 2, L]))
            daf = pbig.tile([128, L], BF, bufs=abufs)
            nc.scalar.activation(daf[:], rep[:, 0, :], AF.Exp,
                                 scale=acol_sb[:, k:k + 1])
            if ab_same:
                dab = daf
            else:
                dab = pbig.tile([128, L], BF, bufs=abufs)
                nc.scalar.activation(dab[:], rep[:, 0, :], AF.Exp,
                                     scale=abcol_sb[:, k:k + 1])
            dbu = pbig.tile([128, L], BF, bufs=abufs)
            deng = nc.vector if (u == 0 and k < 2) else nc.gpsimd
            deng.tensor_tensor(dbu[:], rep[:, 1, :], st["brep"][:],
                               OP.mult)
            st[("ad", k)] = (daf, dab, dbu)

        def stage_b(u, k):
            st = state[u]
            daf, dab, dbu = st.pop(("ad", k))
            hf = pbig.tile([128, L], BF, bufs=3)
            nc.vector.tensor_tensor_scan(hf[:], daf[:], dbu[:], 0.0,
                                         OP.mult, OP.add)
            hb = pbig.tile([128, L], BF, bufs=3)
            nc.vector.tensor_tensor_scan(hb[:], dab[:, ::-1], dbu[:, ::-1],
                                         0.0, OP.mult, OP.add)
            hs = pbig.tile([128, L], BF, bufs=3)
            nc.vector.tensor_tensor(hs[:], hf[:], hb[:, ::-1], OP.add)
            st[("hs", k)] = hs

        def stage_c(u, k, pys):
            st = state[u]
            hs = st.pop(("hs", k))
            hc = pbig.tile([128, L], BF, bufs=3)
            on_dve = (k % 2 == 0) or (u == NU - 1 and k == NK - 1)
            eng = nc.vector if on_dve else nc.gpsimd
            eng.tensor_tensor(hc[:], hs[:], st["crep"][:], OP.mult)
            kk = k % 4
            for lc in range(NLC):
                sl = slice(lc * LC, (lc + 1) * LC)
                nc.tensor.matmul(pys[lc][:], seln_sb[:, kk, :], hc[:, sl],
                                 start=(kk == 0), stop=False)
            if kk == 3:
                kg = k // 4
                xc0 = st["xc"][0]
                y_sb = st["y"]
                for lc in range(NLC):
                    sl = slice(lc * LC, (lc + 1) * LC)
                    nc.tensor.matmul(pys[lc][:],
                                     ddiag_sb[:, 32 * kg:32 * kg + 32],
                                     xc0[0:DH, sl], start=False, stop=True)
                    nc.scalar.activation(y_sb[32 * kg:32 * kg + 32, sl],
                                         pys[lc][:], AF.Copy)

        def emit_out(u, t0, t1, rows=DH, final=False):
            st = state[u]
            xc0 = st["xc"][0]
            y_sb = st["y"]
            for t8 in range(t0, t1):
                sl = slice(t8 * 128, (t8 + 1) * 128)
                if final and t8 % 2 == 1:
                    po = ppy.tile([128, LC], F32, tag="pys", name="ps_of",
                                  bufs=4)
                else:
                    po = ppa.tile([128, LC], F32, tag="ps_o", name="ps_o")
                nc.tensor.matmul(po[:, 0:DIM], y_sb[0:rows, sl],
                                 wouty_sb[0:rows, :], start=True, stop=False)
                nc.tensor.matmul(po[:, 0:DIM], xc0[0:DH, sl], woutz_sb[:],
                                 start=False, stop=True)
                osb = pout.tile([128, DIM], F32)
                if final and t8 % 2 == 1:
                    nc.vector.tensor_scalar_add(osb[:], po[:, 0:DIM], 0.0)
                else:
                    nc.scalar.activation(osb[:], po[:, 0:DIM], AF.Copy)
                nc.sync.dma_start(
                    out_d[u, t8 * 128:(t8 + 1) * 128, :], osb[:])

        def emit_out_tail(u, t0, t1):
            # kg2 rows of y only; host adds this to the partial from emit_out
            st = state[u]
            y_sb = st["y"]
            for t8 in range(t0, t1):
                sl = slice(t8 * 128, (t8 + 1) * 128)
                po = ppa.tile([128, LC], F32, tag="ps_o", name="ps_o")
                nc.tensor.matmul(po[:, 0:DIM], y_sb[64:DH, sl],
                                 wouty_sb[64:DH, :], start=True, stop=True)
                osb = pout.tile([128, DIM], F32, name="osb")
                nc.scalar.activation(osb[:], po[:, 0:DIM], AF.Copy)
                nc.sync.dma_start(
                    outt_d[t8 * 128:(t8 + 1) * 128, :], osb[:])

        # ---- global software-pipelined tile stream ----
        emit_pre0()
        pys = None
        for t in range(NT + SKEW_C):
            if t < NT:
                stage_a(t // NK, t % NK)
            if SKEW_B <= t < NT + SKEW_B:
                tb = t - SKEW_B
                stage_b(tb // NK, tb % NK)
            if t >= SKEW_C:
                tcg = t - SKEW_C
                if tcg % 4 == 0:
                    pys = [ppy.tile([32, LC], F32, tag="pys", name="pys")
                           for _ in range(NLC)]
                stage_c(tcg // NK, tcg % NK, pys)
            u, k = t // NK, t % NK
            if 1 <= k <= 4 and u + 1 < NU:
                emit_xt_chunk(u + 1, k - 1)
            if k == 6 and u + 1 < NU:
                emit_conv(u + 1)
            if k == 7 and u + 1 < NU:
                emit_dt_lc(u + 1, 0)
                emit_dt_lc(u + 1, 1)
            if k == 8 and u + 1 < NU:
                emit_dt_lc(u + 1, 2)
                emit_dt_lc(u + 1, 3)
            if k == 10 and u + 1 < NU:
                emit_cr(u + 1)
            if t == 1:
                emit_cr(0)
            if u >= 1 and 2 <= k <= 3:
                emit_out(u - 1, 8 * (k - 2), 8 * (k - 1))
        emit_out(NU - 1, 0, L // 128, final=True)

    nc.compile()
    return nc


def _get_nc(ab_same: bool):
    if ab_same not in _NC_CACHE:
        _NC_CACHE[ab_same] = _build(ab_same)
    return _NC_CACHE[ab_same]


def _prep_weights(h, in_proj_w, in_proj_b, conv_w, conv_b, A_log, Ab_log, D,
                  x_proj_w, dt_proj_w, dt_proj_b, out_proj_w):
    Gg = np.arange(96 * h, 96 * h + 96)
    perm = np.concatenate([Gg, np.setdiff1d(np.arange(DIN), Gg)])
    f32 = np.float32
    W_in = in_proj_w.astype(f32)
    M = np.empty((3, DIN, DIM), f32)
    bconv = np.empty((3, DIN), f32)
    for k in range(3):
        M[k] = (conv_w[:, 0, k][:, None] * W_in[0::2, :]
                + conv_w[:, 1, k][:, None] * W_in[1::2, :])
        bconv[k] = (conv_w[:, 0, k] * in_proj_b[0::2]
                    + conv_w[:, 1, k] * in_proj_b[1::2])
    Mp = M[:, perm, :]
    import ml_dtypes
    wm = np.empty((9, 128, DIN), ml_dtypes.bfloat16)
    for s in range(3):
        for kt in range(3):
            wm[s * 3 + kt] = Mp[s][:, kt * 128:(kt + 1) * 128].T.astype(
                ml_dtypes.bfloat16)
    bias_int = (bconv.sum(0) + conv_b)[perm]
    efix = np.stack([-bconv[0][perm], -bconv[2][perm]])[None].astype(f32)
    bsil = np.zeros((128, 2), f32)
    bsil[:, 0] = bias_int[:128]
    bsil[0:64, 1] = bias_int[128:]
    A = (-np.exp(A_log)).astype(f32)
    Ab = (-np.exp(Ab_log)).astype(f32)
    xpT = x_proj_w.T[perm]   # (192, 56) contract rows in perm order
    seln = np.zeros((4, 128, 32), f32)
    for v in range(4):
        for r in range(128):
            seln[v, r, 8 * v + r // 16] = 1.0
    return dict(
        wm=wm,
        efix=efix,
        one=np.ones((1, 1), f32),
        bsil=bsil,
        wdd=(xpT[:, 0:DTR].astype(np.float64)
             @ dt_proj_w[Gg].T.astype(np.float64))[0:128].astype(f32),
        wdd2b=(xpT[:, 0:DTR].astype(np.float64)
               @ dt_proj_w[Gg].T.astype(np.float64))[128:192].astype(
                   ml_dtypes.bfloat16),
        wxpbc=np.concatenate(
            [xpT[:, 24 + (np.arange(128) % 16)],
             xpT[:, 40 + (np.arange(128) % 16)]], axis=1).astype(f32).copy(),
        wxpbc2b=np.concatenate(
            [xpT[128:192, 24 + (np.arange(128) % 16)],
             xpT[128:192, 40 + (np.arange(128) % 16)]],
            axis=1).astype(ml_dtypes.bfloat16),
        bsp=dt_proj_b[Gg].reshape(DH, 1).astype(f32),
        acol=A[Gg].reshape(-1).copy(),
        abcol=Ab[Gg].reshape(-1).copy(),
        seln=seln.astype(ml_dtypes.bfloat16),
        ddiag=np.diag(2.0 * D[Gg]).astype(f32),
        wouty=out_proj_w[:, Gg].T.astype(f32).copy(),
        woutz=out_proj_w[:, 192 + Gg].T.astype(f32).copy(),
    )


def kernel(x, in_proj_w, in_proj_b, conv_w, conv_b, A_log, Ab_log, D,
           x_proj_w, dt_proj_w, dt_proj_b, out_proj_w, out_proj_b):
    ab_same = bool(np.array_equal(A_log, Ab_log))
    x = np.asarray(x, np.float32)

    wargs = (in_proj_w, in_proj_b, conv_w, conv_b, A_log, Ab_log, D,
             x_proj_w, dt_proj_w, dt_proj_b, out_proj_w)
    weights = [_prep_weights(h, *[np.asarray(a, np.float32) for a in wargs])
               for h in range(2)]

    in_maps = []
    for core in range(NCORES):
        g, h = divmod(core, 2)
        import ml_dtypes
        xtp = np.zeros((NU, 3, 128, L + 2), ml_dtypes.bfloat16)
        for u in range(NU):
            xs = x[g, u * L:(u + 1) * L, :]        # (L, 384)
            xT = np.ascontiguousarray(xs.T)        # (384, L)
            xtp[u, :, :, 1:L + 1] = xT.reshape(3, 128, L).astype(
                ml_dtypes.bfloat16)
        m = dict(weights[h])
        m["xtp"] = xtp
        in_maps.append(m)

    nc_prog = _get_nc(ab_same)
    r = run_bass_kernel_spmd(nc_prog, in_maps, list(range(NCORES)))
    res = r.results

    out = np.empty((B, SEQ, DIM), np.float32)
    bo = np.asarray(out_proj_b, np.float32)
    for g in range(B):
        for u in range(NU):
            part = (res[2 * g]["out"][u] + res[2 * g + 1]["out"][u] + bo)
            out[g, u * L:(u + 1) * L, :] = part
    return out


# revision 38
# speedup vs baseline: 1.0312x; 1.0312x over previous
"""BiMamba layer Trainium2 kernel (8 NeuronCores, SPMD).

Sharding: 4 batch-groups x 2 d_inner-halves. Core (g, h) handles the 3
(b*f) scan units of batch g for d_inner channels [96h, 96h+96), both scan
directions. Each core emits a partial out-projection; the host sums the
two halves per batch and adds out_proj_b.

Channel permutation: conv/in_proj output channels are permuted host-side
so the core's own 96 channels occupy xc0 rows 0:96 — u is a slice of the
conv output, no duplicate conv group.

All 36 (d,n)-row-tiles stream through one software-pipelined loop with a
2-deep skew so no engine queue head-blocks on another engine:
  A(t): delta/du bcast DMA (HWDGE) -> daf=Exp (ACT) -> dbu (Pool TT)
  B(t-1): fwd/bwd scans + hs=hf+rev(hb) (DVE)
  C(t-2): hc=hs*C (DVE even / Pool odd) -> n-reduction matmuls (PE)
Unit preambles (conv+in_proj+silu, x_proj, dt_proj+softplus, du) and the
previous unit's out-projection are emitted at fixed hooks inside the
stream; activations are batched by act-table set to avoid reloads.
"""
from contextlib import ExitStack

import numpy as np

import concourse.bass as bass
import concourse.tile as tile
from concourse import bacc, mybir
from concourse.bass_utils import run_bass_kernel_spmd

F32 = mybir.dt.float32
FR = mybir.dt.float32r
BF = mybir.dt.bfloat16
AF = mybir.ActivationFunctionType
OP = mybir.AluOpType

B, SEQ, DIM = 4, 6144, 384
L = 2048                  # per-unit sequence length
NU = 3                    # units per core
DIN, DH, NST, DTR = 192, 96, 16, 24
NK = 12                   # (DH*NST)//128 row-tiles
LC = 512                  # psum column chunk
NLC = L // LC
NCORES = 8
NT = NU * NK              # global tile count
SKEW_B, SKEW_C = 1, 2

_NC_CACHE = {}


def _patch_act_tables():
    """Confine Exp/Ln to the combined natural_log_exp table so the act-table
    chooser never alternates between the separate exp / ln tables."""
    import concourse.hw_specs as hw_specs
    if getattr(hw_specs.get_activation_tables, "_bimamba_patched", False):
        return
    orig = hw_specs.get_activation_tables

    def patched(module_arch):
        tables = dict(orig(module_arch))
        import concourse.mybir as mb
        aft = mb.ActivationFunctionType
        for name, funcs in tables.items():
            if name != "natural_log_exp_and_others":
                tables[name] = funcs - {aft.Exp, aft.Ln}
        return tables

    patched._bimamba_patched = True
    hw_specs.get_activation_tables = patched
    import concourse.bacc as bacc_mod
    bacc_mod.get_activation_tables = patched


def _build(ab_same: bool):
    _patch_act_tables()
    nc = bacc.Bacc("TRN2", target_bir_lowering=False, debug=False)

    def din(name, shape, dt=F32):
        return nc.dram_tensor(name, list(shape), dt, kind="ExternalInput").ap()

    xtp_d = din("xtp", (NU, 3, 128, L + 2), BF)
    wm_d = din("wm", (9, 128, DIN), BF)
    efix_d = din("efix", (1, 2, DIN))
    one_d = din("one", (1, 1))
    bsil_d = din("bsil", (128, 2))
    wdd_d = din("wdd", (128, DH))
    wdd2b_d = din("wdd2b", (64, DH), BF)
    wxpbc_d = din("wxpbc", (DIN, 256))
    wxpbc2b_d = din("wxpbc2b", (64, 256), BF)
    bsp_d = din("bsp", (DH, 1))
    acol_d = din("acol", (NK * 128,))
    abcol_d = din("abcol", (NK * 128,))
    seln_d = din("seln", (4, 128, 32), BF)
    ddiag_d = din("ddiag", (DH, DH))
    wouty_d = din("wouty", (DH, DIM))
    woutz_d = din("woutz", (DH, DIM))
    out_d = nc.dram_tensor("out", [NU, L, DIM], F32, kind="ExternalOutput").ap()
    outt_d = nc.dram_tensor("out_tail", [L, DIM], F32,
                            kind="ExternalOutput").ap()

    abufs = 4 if ab_same else 2

    with tile.TileContext(nc) as tc, ExitStack() as ctx:
        cp = ctx.enter_context(tc.tile_pool(name="consts", bufs=1))
        px = ctx.enter_context(tc.tile_pool(name="px", bufs=1))
        pxc = ctx.enter_context(tc.tile_pool(name="pxc", bufs=2))
        psm = ctx.enter_context(tc.tile_pool(name="psm", bufs=2))
        psm1 = ctx.enter_context(tc.tile_pool(name="psm1", bufs=1))
        pbig = ctx.enter_context(tc.tile_pool(name="pbig", bufs=2))
        pout = ctx.enter_context(tc.tile_pool(name="pout", bufs=4))
        ppa = ctx.enter_context(tc.tile_pool(name="ppa", bufs=2, space="PSUM"))
        ppy = ctx.enter_context(tc.tile_pool(name="ppy", bufs=4, space="PSUM"))

        state = {}

        def emit_xt_chunk(u, c):
            if c == 0:
                state[u] = {"xt": px.tile([128, 3, L + 2], BF, name="xt")}
            xt = state[u]["xt"]
            c0 = c * LC
            c1 = min(L + 2, c0 + LC + 2)
            nc.sync.dma_start(
                xt[:, :, c0:c1],
                xtp_d[u][:, :, c0:c1].transpose([1, 0, 2]))

        # ---- xt(0) first, then constants ordered by first use ----
        warm = cp.tile([1, 2], F32)
        nc.gpsimd.memset(warm[:], 0.0)
        nc.gpsimd.tensor_tensor(warm[:, 0:1], warm[:, 0:1], warm[:, 1:2],
                                OP.mult)
        wps = ppa.tile([1, 64], F32, tag="ppa", name="wps")
        for i in range(20):
            nc.tensor.matmul(wps[:], warm[0:1, 0:1], warm[0:1, 0:1]
                             .broadcast_to([1, 64]), start=(i == 0),
                             stop=(i == 19))
        for c in range(NLC):
            emit_xt_chunk(0, c)
        wm_sb = cp.tile([128, 9, DIN], BF)
        nc.sync.dma_start(wm_sb[:], wm_d.transpose([1, 0, 2]))
        efix_sb = cp.tile([1, 2, DIN], F32)
        nc.sync.dma_start(efix_sb[:], efix_d)
        one_sb = cp.tile([1, 1], F32)
        nc.sync.dma_start(one_sb[:], one_d)
        bsil_sb = cp.tile([128, 2], F32)
        nc.sync.dma_start(bsil_sb[:], bsil_d)
        wdd_sb = cp.tile([128, DH], FR)
        nc.sync.dma_start(wdd_sb[:], wdd_d.bitcast(FR))
        wdd2_sb = cp.tile([64, DH], BF)
        nc.sync.dma_start(wdd2_sb[:], wdd2b_d)
        bsp_sb = cp.tile([DH, 1], F32)
        nc.sync.dma_start(bsp_sb[:], bsp_d)
        wxpbc_sb = cp.tile([128, 256], FR)
        nc.sync.dma_start(wxpbc_sb[:], wxpbc_d[0:128, :].bitcast(FR))
        wxpbc2_sb = cp.tile([64, 256], BF)
        nc.sync.dma_start(wxpbc2_sb[:], wxpbc2b_d)
        acol_sb = cp.tile([128, NK], F32)
        nc.sync.dma_start(acol_sb[:], acol_d.rearrange("(k p) -> p k", p=128))
        abcol_sb = cp.tile([128, NK], F32)
        nc.sync.dma_start(abcol_sb[:], abcol_d.rearrange("(k p) -> p k", p=128))
        seln_sb = cp.tile([128, 4, 32], BF)
        nc.sync.dma_start(seln_sb[:], seln_d.transpose([1, 0, 2]))
        ddiag_sb = cp.tile([DH, DH], FR)
        nc.sync.dma_start(ddiag_sb[:], ddiag_d.bitcast(FR))
        wouty_sb = cp.tile([DH, DIM], FR)
        nc.sync.dma_start(wouty_sb[:], wouty_d.bitcast(FR))
        woutz_sb = cp.tile([DH, DIM], FR)
        nc.sync.dma_start(woutz_sb[:], woutz_d.bitcast(FR))

        def alloc_unit(u):
            st = state[u]
            st["xc"] = (pxc.tile([128, L], FR, name="xc0"),
                        pxc.tile([64, L], BF, name="xc1", bufs=1))
            st["brep"] = psm.tile([128, L], BF, name="brep")
            st["crep"] = psm.tile([128, L], BF, name="crep")
            st["ddu"] = psm.tile([DH, 2, L], BF, name="ddu")
            st["ubf"] = psm.tile([DH, L], BF, name="ubf", bufs=1)
            st["y"] = psm.tile([DH, L], FR, name="y_sb")

        def emit_conv_lc(u, lc, ctag="ppa", cbufs=2):
            xc0, xc1 = state[u]["xc"]
            xt = state[u]["xt"]
            sl = slice(lc * LC, (lc + 1) * LC)
            for c0, cw, dst, bias_ap in [
                    (0, 128, xc0, bsil_sb[0:128, 0:1]),
                    (128, 64, xc1, bsil_sb[0:64, 1:2])]:
                pool = ppy if ctag == "pys" else ppa
                ps = pool.tile([128, LC], F32, tag=ctag, name="ps_conv",
                               bufs=cbufs)
                mms = []
                for s in range(3):
                    for kt in range(3):
                        mms.append((ps[0:cw, :],
                                    wm_sb[:, s * 3 + kt, c0:c0 + cw],
                                    xt[:, kt, s + lc * LC:s + lc * LC + LC]))
                if lc == 0:
                    mms.append((ps[0:cw, 0:1],
                                efix_sb[0:1, 0, c0:c0 + cw], one_sb[:]))
                if lc == NLC - 1:
                    mms.append((ps[0:cw, LC - 1:LC],
                                efix_sb[0:1, 1, c0:c0 + cw], one_sb[:]))
                for i, (o, lh, rh) in enumerate(mms):
                    nc.tensor.matmul(o, lh, rh, start=(i == 0),
                                     stop=(i == len(mms) - 1))
                nc.scalar.activation(dst[:, sl], ps[0:cw, :], AF.Silu,
                                     bias=bias_ap)

        def emit_dt_lc(u, lc):
            """dt x_proj + dt_proj + softplus + du + B for one chunk."""
            st = state[u]
            xc0, xc1 = st["xc"]
            sl = slice(lc * LC, (lc + 1) * LC)
            pdp = ppa.tile([128, LC], F32, tag="ppa", name="ps_dp")
            nc.tensor.matmul(pdp[0:DH, :], wdd_sb[:], xc0[:, sl],
                             start=True, stop=False)
            nc.tensor.matmul(pdp[0:DH, :], wdd2_sb[:], xc1[:, sl],
                             start=False, stop=True)
            nc.scalar.activation(st["ddu"][:, 1, sl], pdp[0:DH, :], AF.Exp,
                                 bias=bsp_sb[:])
            nc.scalar.activation(st["ddu"][:, 0, sl], st["ddu"][:, 1, sl],
                                 AF.Ln, bias=1.0)
            nc.scalar.activation(st["ubf"][:, sl], st["xc"][0][0:DH, sl],
                                 AF.Copy)
            nc.vector.tensor_tensor(st["ddu"][:, 1, sl], st["ddu"][:, 0, sl],
                                    st["ubf"][:, sl], OP.mult)
            pbr = ppa.tile([128, LC], F32, tag="ps_o", name="ps_br")
            nc.tensor.matmul(pbr[:], wxpbc_sb[:, 0:128], xc0[:, sl],
                             start=True, stop=False)
            nc.tensor.matmul(pbr[:], wxpbc2_sb[:, 0:128], xc1[:, sl],
                             start=False, stop=True)
            nc.scalar.activation(st["brep"][:, sl], pbr[:], AF.Copy)

        def emit_cr(u):
            st = state[u]
            xc0, xc1 = st["xc"]
            for lc in range(NLC):
                sl = slice(lc * LC, (lc + 1) * LC)
                pcr = ppa.tile([128, LC], F32, tag="ppa", name="ps_cr")
                nc.tensor.matmul(pcr[:], wxpbc_sb[:, 128:256], xc0[:, sl],
                                 start=True, stop=False)
                nc.tensor.matmul(pcr[:], wxpbc2_sb[:, 128:256], xc1[:, sl],
                                 start=False, stop=True)
                nc.scalar.activation(st["crep"][:, sl], pcr[:], AF.Copy)

        def emit_pre0():
            alloc_unit(0)
            emit_conv_lc(0, 0, ctag="pys", cbufs=4)
            emit_conv_lc(0, 1, ctag="pys", cbufs=4)
            emit_dt_lc(0, 0)
            emit_conv_lc(0, 2, ctag="pys", cbufs=4)
            emit_dt_lc(0, 1)
            emit_conv_lc(0, 3, ctag="pys", cbufs=4)
            emit_dt_lc(0, 2)
            emit_dt_lc(0, 3)

        def emit_conv(u):
            alloc_unit(u)
            for lc in range(NLC):
                emit_conv_lc(u, lc, ctag=("ppa" if lc % 2 == 0 else "ps_o"))



        def stage_a(u, k):
            st = state[u]
            rep = pbig.tile([128, 2, L], BF, bufs=3)
            nc.sync.dma_start(
                rep[:],
                st["ddu"][8 * k:8 * k + 8, :, :].unsqueeze(1)
                .broadcast_to([8, 16, 2, L]))
            daf = pbig.tile([128, L], BF, bufs=abufs)
            nc.scalar.activation(daf[:], rep[:, 0, :], AF.Exp,
                                 scale=acol_sb[:, k:k + 1])
            if ab_same:
                dab = daf
            else:
                dab = pbig.tile([128, L], BF, bufs=abufs)
                nc.scalar.activation(dab[:], rep[:, 0, :], AF.Exp,
                                     scale=abcol_sb[:, k:k + 1])
            dbu = pbig.tile([128, L], BF, bufs=abufs)
            deng = nc.vector if (u == 0 and k < 2) else nc.gpsimd
            deng.tensor_tensor(dbu[:], rep[:, 1, :], st["brep"][:],
                               OP.mult)
            st[("ad", k)] = (daf, dab, dbu)

        def stage_b(u, k):
            st = state[u]
            daf, dab, dbu = st.pop(("ad", k))
            hf = pbig.tile([128, L], BF, bufs=3)
            nc.vector.tensor_tensor_scan(hf[:], daf[:], dbu[:], 0.0,
                                         OP.mult, OP.add)
            hb = pbig.tile([128, L], BF, bufs=3)
            nc.vector.tensor_tensor_scan(hb[:], dab[:, ::-1], dbu[:, ::-1],
                                         0.0, OP.mult, OP.add)
            hs = pbig.tile([128, L], BF, bufs=3)
            nc.vector.tensor_tensor(hs[:], hf[:], hb[:, ::-1], OP.add)
            st[("hs", k)] = hs

        def stage_c(u, k, pys):
            st = state[u]
            hs = st.pop(("hs", k))
            hc = pbig.tile([128, L], BF, bufs=3)
            on_dve = (k % 2 == 0) or (u == NU - 1 and k == NK - 1)
            eng = nc.vector if on_dve else nc.gpsimd
            eng.tensor_tensor(hc[:], hs[:], st["crep"][:], OP.mult)
            kk = k % 4
            for lc in range(NLC):
                sl = slice(lc * LC, (lc + 1) * LC)
                nc.tensor.matmul(pys[lc][:], seln_sb[:, kk, :], hc[:, sl],
                                 start=(kk == 0), stop=False)
            if kk == 3:
                kg = k // 4
                xc0 = st["xc"][0]
                y_sb = st["y"]
                for lc in range(NLC):
                    sl = slice(lc * LC, (lc + 1) * LC)
                    nc.tensor.matmul(pys[lc][:],
                                     ddiag_sb[:, 32 * kg:32 * kg + 32],
                                     xc0[0:DH, sl], start=False, stop=True)
                    nc.scalar.activation(y_sb[32 * kg:32 * kg + 32, sl],
                                         pys[lc][:], AF.Copy)

        def emit_out(u, t0, t1, rows=DH, final=False):
            st = state[u]
            xc0 = st["xc"][0]
            y_sb = st["y"]
            for t8 in range(t0, t1):
                sl = slice(t8 * 128, (t8 + 1) * 128)
                if final and t8 % 2 == 1:
                    po = ppy.tile([128, LC], F32, tag="pys", name="ps_of",
                                  bufs=4)
                else:
                    po = ppa.tile([128, LC], F32, tag="ps_o", name="ps_o")
                nc.tensor.matmul(po[:, 0:DIM], y_sb[0:rows, sl],
                                 wouty_sb[0:rows, :], start=True, stop=False)
                nc.tensor.matmul(po[:, 0:DIM], xc0[0:DH, sl], woutz_sb[:],
                                 start=False, stop=True)
                osb = pout.tile([128, DIM], F32)
                if final and t8 % 2 == 1:
                    nc.vector.tensor_scalar_add(osb[:], po[:, 0:DIM], 0.0)
                else:
                    nc.scalar.activation(osb[:], po[:, 0:DIM], AF.Copy)
                nc.sync.dma_start(
                    out_d[u, t8 * 128:(t8 + 1) * 128, :], osb[:])

        def emit_out_tail(u, t0, t1):
            # kg2 rows of y only; host adds this to the partial from emit_out
            st = state[u]
            y_sb = st["y"]
            for t8 in range(t0, t1):
                sl = slice(t8 * 128, (t8 + 1) * 128)
                po = ppa.tile([128, LC], F32, tag="ps_o", name="ps_o")
                nc.tensor.matmul(po[:, 0:DIM], y_sb[64:DH, sl],
                                 wouty_sb[64:DH, :], start=True, stop=True)
                osb = pout.tile([128, DIM], F32, name="osb")
                nc.scalar.activation(osb[:], po[:, 0:DIM], AF.Copy)
                nc.sync.dma_start(
                    outt_d[t8 * 128:(t8 + 1) * 128, :], osb[:])

        # ---- global software-pipelined tile stream ----
        emit_pre0()
        pys = None
        for t in range(NT + SKEW_C):
            if t < NT:
                stage_a(t // NK, t % NK)
            if SKEW_B <= t < NT + SKEW_B:
                tb = t - SKEW_B
                stage_b(tb // NK, tb % NK)
            if t >= SKEW_C:
                tcg = t - SKEW_C
                if tcg % 4 == 0:
                    pys = [ppy.tile([32, LC], F32, tag="pys", name="pys")
                           for _ in range(NLC)]
                stage_c(tcg // NK, tcg % NK, pys)
            u, k = t // NK, t % NK
            if 1 <= k <= 4 and u + 1 < NU:
                emit_xt_chunk(u + 1, k - 1)
            if k == 6 and u + 1 < NU:
                emit_conv(u + 1)
            if k == 7 and u + 1 < NU:
                emit_dt_lc(u + 1, 0)
                emit_dt_lc(u + 1, 1)
            if k == 8 and u + 1 < NU:
                emit_dt_lc(u + 1, 2)
                emit_dt_lc(u + 1, 3)
            if k == 10 and u + 1 < NU:
                emit_cr(u + 1)
            if t == 1:
                emit_cr(0)
            if u >= 1 and 2 <= k <= 3:
                emit_out(u - 1, 8 * (k - 2), 8 * (k - 1))
        emit_out(NU - 1, 0, L // 128, final=True)

    nc.compile()
    return nc


def _get_nc(ab_same: bool):
    if ab_same not in _NC_CACHE:
        _NC_CACHE[ab_same] = _build(ab_same)
    return _NC_CACHE[ab_same]


def _prep_weights(h, in_proj_w, in_proj_b, conv_w, conv_b, A_log, Ab_log, D,
                  x_proj_w, dt_proj_w, dt_proj_b, out_proj_w):
    Gg = np.arange(96 * h, 96 * h + 96)
    perm = np.concatenate([Gg, np.setdiff1d(np.arange(DIN), Gg)])
    f32 = np.float32
    W_in = in_proj_w.astype(f32)
    M = np.empty((3, DIN, DIM), f32)
    bconv = np.empty((3, DIN), f32)
    for k in range(3):
        M[k] = (conv_w[:, 0, k][:, None] * W_in[0::2, :]
                + conv_w[:, 1, k][:, None] * W_in[1::2, :])
        bconv[k] = (conv_w[:, 0, k] * in_proj_b[0::2]
                    + conv_w[:, 1, k] * in_proj_b[1::2])
    Mp = M[:, perm, :]
    import ml_dtypes
    wm = np.empty((9, 128, DIN), ml_dtypes.bfloat16)
    for s in range(3):
        for kt in range(3):
            wm[s * 3 + kt] = Mp[s][:, kt * 128:(kt + 1) * 128].T.astype(
                ml_dtypes.bfloat16)
    bias_int = (bconv.sum(0) + conv_b)[perm]
    efix = np.stack([-bconv[0][perm], -bconv[2][perm]])[None].astype(f32)
    bsil = np.zeros((128, 2), f32)
    bsil[:, 0] = bias_int[:128]
    bsil[0:64, 1] = bias_int[128:]
    A = (-np.exp(A_log)).astype(f32)
    Ab = (-np.exp(Ab_log)).astype(f32)
    xpT = x_proj_w.T[perm]   # (192, 56) contract rows in perm order
    seln = np.zeros((4, 128, 32), f32)
    for v in range(4):
        for r in range(128):
            seln[v, r, 8 * v + r // 16] = 1.0
    return dict(
        wm=wm,
        efix=efix,
        one=np.ones((1, 1), f32),
        bsil=bsil,
        wdd=(xpT[:, 0:DTR].astype(np.float64)
             @ dt_proj_w[Gg].T.astype(np.float64))[0:128].astype(f32),
        wdd2b=(xpT[:, 0:DTR].astype(np.float64)
               @ dt_proj_w[Gg].T.astype(np.float64))[128:192].astype(
                   ml_dtypes.bfloat16),
        wxpbc=np.concatenate(
            [xpT[:, 24 + (np.arange(128) % 16)],
             xpT[:, 40 + (np.arange(128) % 16)]], axis=1).astype(f32).copy(),
        wxpbc2b=np.concatenate(
            [xpT[128:192, 24 + (np.arange(128) % 16)],
             xpT[128:192, 40 + (np.arange(128) % 16)]],
            axis=1).astype(ml_dtypes.bfloat16),
        bsp=dt_proj_b[Gg].reshape(DH, 1).astype(f32),
        acol=A[Gg].reshape(-1).copy(),
        abcol=Ab[Gg].reshape(-1).copy(),
        seln=seln.astype(ml_dtypes.bfloat16),
        ddiag=np.diag(2.0 * D[Gg]).astype(f32),
        wouty=out_proj_w[:, Gg].T.astype(f32).copy(),
        woutz=out_proj_w[:, 192 + Gg].T.astype(f32).copy(),
    )


def kernel(x, in_proj_w, in_proj_b, conv_w, conv_b, A_log, Ab_log, D,
           x_proj_w, dt_proj_w, dt_proj_b, out_proj_w, out_proj_b):
    ab_same = bool(np.array_equal(A_log, Ab_log))
    x = np.asarray(x, np.float32)

    wargs = (in_proj_w, in_proj_b, conv_w, conv_b, A_log, Ab_log, D,
             x_proj_w, dt_proj_w, dt_proj_b, out_proj_w)
    weights = [_prep_weights(h, *[np.asarray(a, np.float32) for a in wargs])
               for h in range(2)]

    in_maps = []
    for core in range(NCORES):
        g, h = divmod(core, 2)
        import ml_dtypes
        xtp = np.zeros((NU, 3, 128, L + 2), ml_dtypes.bfloat16)
        for u in range(NU):
            xs = x[g, u * L:(u + 1) * L, :]        # (L, 384)
            xT = np.ascontiguousarray(xs.T)        # (384, L)
            xtp[u, :, :, 1:L + 1] = xT.reshape(3, 128, L).astype(
                ml_dtypes.bfloat16)
        m = dict(weights[h])
        m["xtp"] = xtp
        in_maps.append(m)

    nc_prog = _get_nc(ab_same)
    r = run_bass_kernel_spmd(nc_prog, in_maps, list(range(NCORES)))
    res = r.results

    out = np.empty((B, SEQ, DIM), np.float32)
    bo = np.asarray(out_proj_b, np.float32)
    for g in range(B):
        for u in range(NU):
            part = (res[2 * g]["out"][u] + res[2 * g + 1]["out"][u] + bo)
            out[g, u * L:(u + 1) * L, :] = part
    return out
